# revision 1
# baseline (speedup 1.0000x reference)
"""AdjustableConvolution2d Trainium2 kernel.

Data-parallel over batch: 8 samples -> 8 NeuronCores, no collectives.

Per-core pipeline (one sample, c=256 channels, 64x64 spatial):
  1. filter logits in one fused matmul: host constant-folds
     W_comb=(Wt@Wf)/100, b_comb=(bt@Wf+bf)/100, device computes
     temp @ W_comb + b_comb (bias via a K=1 accumulate row), then
     softmax over the 9 taps laid out as per-partition scalars.
  2. depthwise 3x3 with per-(sample,channel) taps: computed on the
     TensorEngine as diag(filt[:,tap]) @ shifted_view(padded_image) in bf16,
     9 taps accumulated in fp32 PSUM.
  3. 1x1 channel combine: WcT chunks as bf16 stationary operand, accumulate
     over channel chunks in PSUM, add bias on ACT/DVE while copying to SBUF.
Host-side prep: layout, bf16 rounding of matmul operands, and constant
folding of the static weight-weight product.
"""

import numpy as np
import ml_dtypes

BS, C, H, W = 8, 256, 64, 64
KK = 3
P = 128
CC = C // P            # channel chunks of 128
HP, WP = H + 2, W + 2  # zero-padded spatial
SQ, TIN = 32, 256
CKK = C * KK * KK      # 2304
RS = 8                 # output rows per hw-slice
NS = RS * W            # 512 elements per hw-slice
NSL = H // RS          # 8 slices
GRP = 4                # hw-slices per psum group (stationary-weight reuse)

# blob_a column layout (fp32 columns, 128 partitions)
A_WCT0, A_WCT1 = 0, 256        # Wc.T as bf16 pairs packed in fp32 words
A_BC0, A_BC1 = 256, 258        # bc [p, cc]
A_TMP = 258                    # temp_feat bf16 pair [p, cc]
A_WT0, A_WT1 = 259, 291       # Wt bf16 pairs [p, cc*32+s]
A_BT = 291                     # bt fp32 in rows 0:32
A_BCB0, A_BCB1 = 292, 310      # bf/100 transposed [p, cc*9+k], fp32
A_N = 310

_CACHE = {}


def _build():
    from contextlib import ExitStack

    import concourse.bass as bass
    import concourse.bacc as bacc
    import concourse.mybir as mybir
    import concourse.tile as tile
    from concourse import masks

    dt = mybir.dt
    f32 = dt.float32
    bf16 = dt.bfloat16
    AF = mybir.ActivationFunctionType
    ALU = mybir.AluOpType
    AX = mybir.AxisListType

    nc = bacc.Bacc(
        "TRN2", target_bir_lowering=False, debug=False, enable_asserts=False
    )

    NF = 512
    NCH = CKK // NF + (1 if CKK % NF else 0)  # 5 chunks of <=512 logits
    img_d = nc.dram_tensor("img", [C, HP * WP], bf16, kind="ExternalInput")
    bla_d = nc.dram_tensor("bla", [P, A_N], f32, kind="ExternalInput")
    wf_d = nc.dram_tensor("wf", [SQ, CKK], bf16, kind="ExternalInput")
    out_d = nc.dram_tensor("out", [C, H, W], f32, kind="ExternalOutput")

    with tile.TileContext(nc) as tc, ExitStack() as ctx:
        constp = ctx.enter_context(tc.tile_pool(name="const", bufs=1))
        imgp = ctx.enter_context(tc.tile_pool(name="img", bufs=1))
        filtp = ctx.enter_context(tc.tile_pool(name="filt", bufs=1))
        sps = ctx.enter_context(
            tc.tile_pool(name="spsum", bufs=2, space=bass.MemorySpace.PSUM)
        )
        midps = ctx.enter_context(
            tc.tile_pool(name="midps", bufs=3, space=bass.MemorySpace.PSUM)
        )
        outps = ctx.enter_context(
            tc.tile_pool(name="outps", bufs=3, space=bass.MemorySpace.PSUM)
        )
        accp = ctx.enter_context(tc.tile_pool(name="accp", bufs=2))
        midsb = ctx.enter_context(tc.tile_pool(name="midsb", bufs=16))
        outsb = ctx.enter_context(tc.tile_pool(name="outsb", bufs=3))

        # ---- weights first on the scalar-engine DMA queue.  The filter-MLP
        # columns (27KB) land before Wc.T (131KB, not needed until the first
        # 1x1 ~10us later), so the filter chain starts ~2us earlier.
        bla = constp.tile([P, A_N], f32)
        nc.scalar.dma_start(bla[:, A_BC0:A_N], bla_d[:, A_BC0:A_N])
        wf = constp.tile([SQ, CKK], bf16)  # Wf/100
        nc.scalar.dma_start(wf[:], wf_d[:, :])
        nc.scalar.dma_start(bla[:, A_WCT0:A_WCT1], bla_d[:, A_WCT0:A_WCT1])

        wct_sb = bla[:, A_WCT0:A_WCT1].bitcast(bf16)  # [128, 512]
        wct_v = wct_sb.rearrange("p (cc o) -> p cc o", cc=CC)
        bc_v = bla[:, A_BC0:A_BC1]
        temp_v = bla[:, A_TMP : A_TMP + 1].bitcast(bf16)  # [128, 2] bf16
        wt_v = (
            bla[:, A_WT0:A_WT1].bitcast(bf16).rearrange("p (cc s) -> p cc s", cc=CC)
        )
        bt_v = bla[:SQ, A_BT : A_BT + 1]  # [32, 1] fp32
        bcbT_v = bla[:, A_BCB0:A_BCB1].rearrange("p (cc k) -> p cc k", cc=CC)

        ident = constp.tile([P, P], f32)
        masks.make_identity(nc, ident[:])
        scratch = constp.tile([P, NF], bf16)
        nc.gpsimd.memset(scratch[:], 0.0)

        img_sb = imgp.tile([P, CC, HP * WP], bf16)
        imgv = []
        for cc in range(CC):
            imgv.append(img_sb[:, cc, :].rearrange("p (r w) -> p r w", w=WP))

        # ---- filter MLP: t = temp@Wt + bt, logits = t@(Wf/100) + bf/100 ----
        # borrow a midps slot: that pool is idle until the main loop, so
        # t_ps stops competing with the f-chunk tiles for the small pool
        t_ps = midps.tile([SQ, 1], f32, name="tps", tag="mid")
        for cc in range(CC):
            nc.tensor.matmul(
                t_ps[:],
                wt_v[:, cc, :],
                temp_v[:, cc : cc + 1],
                start=(cc == 0),
                stop=(cc == CC - 1),
            )
        t_sb = filtp.tile([SQ, 1], bf16)
        nc.scalar.activation(t_sb[:], t_ps[:], AF.Identity, bias=bt_v)

        flt_sb = filtp.tile([1, CKK], f32)
        for j in range(NCH):
            c0 = j * NF
            n = min(NF, CKK - c0)
            f_ps = sps.tile([1, NF], f32, name="fps", tag="small")
            nc.tensor.matmul(f_ps[:, :n], t_sb[:], wf[:, c0 : c0 + n])
            nc.vector.tensor_copy(flt_sb[:, c0 : c0 + n], f_ps[:, :n])

        # warm-keeper matmuls: keep the PE busy (and the HAM clock-gate
        # open) while the softmax chain resolves; outputs are never read
        for _ in range(15):
            j_ps = sps.tile([P, NF], f32, name="jps", tag="small")
            nc.tensor.matmul(j_ps[:], scratch[:, :P], scratch[:])

        # image DMAs here: ACT descriptors queue after t_sb but before exp,
        # so transfers overlap the filter chain and finish before the diag
        # matmuls need them; first rows of both chunks land first
        HSPLIT = 35 * WP
        for lo, hi in ((0, HSPLIT), (HSPLIT, HP * WP)):
            for cc in range(CC):
                nc.scalar.dma_start(
                    img_sb[:, cc, lo:hi], img_d[cc * P : (cc + 1) * P, lo:hi]
                )

        # per-chunk pipeline: scatter -> softmax -> diag, cc0 first so the
        # TensorEngine starts as early as possible
        fT = filtp.tile([P, CC, KK * KK], f32)
        fTb = filtp.tile([P, CC, KK * KK], f32)
        e = filtp.tile([P, CC, KK * KK], f32)
        s = filtp.tile([P, CC], f32)
        r = filtp.tile([P, CC], f32)
        diag = constp.tile([P, CC, KK * KK, P], bf16)
        filtn1 = filtp.tile([P, KK * KK], f32)
        for cc in range(CC):
            nc.sync.dma_start(
                fT[:, cc, :],
                flt_sb[:, cc * P * KK * KK : (cc + 1) * P * KK * KK].rearrange(
                    "one (p k) -> one p k", k=KK * KK
                ),
            )
            nc.vector.scalar_tensor_tensor(
                fTb[:, cc],
                fT[:, cc, :],
                1.0,
                bcbT_v[:, cc],
                op0=ALU.mult,
                op1=ALU.add,
            )
            nc.scalar.activation(e[:, cc], fTb[:, cc], AF.Exp)
            nc.vector.reduce_sum(s[:, cc : cc + 1], e[:, cc], axis=AX.X)
            nc.vector.reciprocal(r[:, cc : cc + 1], s[:, cc : cc + 1])
            if cc == 1:
                nc.vector.tensor_scalar_mul(
                    filtn1[:], e[:, 1], r[:, 1:2]
                )
            # diag = ident * e * (1/sum) fused in one op per split
            splits = ((0, 3), (3, 9)) if cc == 0 else ((0, 9),)
            for lo, hi in splits:
                nc.vector.scalar_tensor_tensor(
                    diag[:, cc, lo:hi],
                    e[:, cc, lo:hi].unsqueeze(2).to_broadcast((P, hi - lo, P)),
                    r[:, cc : cc + 1],
                    ident[:, :].unsqueeze(1).to_broadcast((P, hi - lo, P)),
                    op0=ALU.mult,
                    op1=ALU.mult,
                )

        # ---- main loop: flat slice pipeline, 1x1 lags one slice ------------
        out_flat = out_d[:, :, :].rearrange("c h w -> c (h w)")
        DVE_SLICES = (2, 3, 5, 7)  # depthwise (cc=1) on DVE for these slices

        def depthwise_pe(cc, hs):
            mt = midps.tile([P, NS], f32, name="mid", tag="mid")
            for t9 in range(KK * KK):
                di, dj = t9 // KK, t9 % KK
                r0 = RS * hs + di
                nc.tensor.matmul(
                    mt[:],
                    diag[:, cc, t9, :],
                    imgv[cc][:, r0 : r0 + RS, dj : dj + W],
                    start=(t9 == 0),
                    stop=(t9 == KK * KK - 1),
                )
            m = midsb.tile([P, NS], bf16, name="midt", tag="midt")
            nc.scalar.copy(m[:], mt[:])
            return m

        def depthwise_dve(hs):
            acc = accp.tile([P, NS], f32, name="dacc", tag="dacc")
            for t9 in range(KK * KK):
                di, dj = t9 // KK, t9 % KK
                rhs_v = imgv[1][:, RS * hs + di : RS * hs + di + RS, dj : dj + W]
                if t9 == 0:
                    nc.vector.tensor_scalar_mul(acc[:], rhs_v, filtn1[:, 0:1])
                else:
                    nc.vector.scalar_tensor_tensor(
                        acc[:],
                        rhs_v,
                        filtn1[:, t9 : t9 + 1],
                        acc[:],
                        op0=ALU.mult,
                        op1=ALU.add,
                    )
            m = midsb.tile([P, NS], bf16, name="midt", tag="midt")
            nc.vector.tensor_copy(m[:], acc[:])
            return m

        def one_by_one(hs, mids_hs):
            for oc in range(CC):
                o_ps = outps.tile([P, NS], f32, name="ops", tag="ops")
                for cc in range(CC):
                    nc.tensor.matmul(
                        o_ps[:],
                        wct_v[:, cc, oc * P : (oc + 1) * P],
                        mids_hs[cc][:],
                        start=(cc == 0),
                        stop=(cc == CC - 1),
                    )
                ob = outsb.tile([P, NS], f32, name="ob", tag="ob")
                on_act = not (hs == NSL - 1 and oc == 0)
                if on_act:
                    nc.scalar.activation(
                        ob[:], o_ps[:], AF.Identity, bias=bc_v[:, oc : oc + 1]
                    )
                else:
                    nc.vector.tensor_scalar_add(
                        ob[:], o_ps[:], bc_v[:, oc : oc + 1]
                    )
                if hs == NSL - 1:
                    hh = NS // 2
                    nc.sync.dma_start(
                        out_flat[oc * P : (oc + 1) * P, hs * NS : hs * NS + hh],
                        ob[:, :hh],
                    )
                    nc.scalar.dma_start(
                        out_flat[oc * P : (oc + 1) * P, hs * NS + hh : (hs + 1) * NS],
                        ob[:, hh:],
                    )
                else:
                    nc.sync.dma_start(
                        out_flat[oc * P : (oc + 1) * P, hs * NS : (hs + 1) * NS],
                        ob[:],
                    )

        prev = None
        for hs in range(NSL):
            m0 = depthwise_pe(0, hs)
            if hs in DVE_SLICES:
                m1 = depthwise_dve(hs)
            else:
                m1 = depthwise_pe(1, hs)
            if prev is not None:
                one_by_one(hs - 1, prev)
            prev = [m0, m1]
        one_by_one(NSL - 1, prev)

    nc.compile()
    return nc


def _get_nc():
    if "nc" not in _CACHE:
        _CACHE["nc"] = _build()
    return _CACHE["nc"]


def _prep_in_maps(image_feat, temp_feat, Wt, bt, Wf, bf, Wc, bc):
    f = lambda a: np.ascontiguousarray(np.asarray(a, dtype=np.float32))
    image_feat = f(image_feat)
    temp_feat = f(temp_feat)

    img_pad = np.zeros((BS, C, HP, WP), ml_dtypes.bfloat16)
    img_pad[:, :, 1 : H + 1, 1 : W + 1] = image_feat.astype(ml_dtypes.bfloat16)
    img_pad = img_pad.reshape(BS, C, HP * WP)

    # fold the softmax temperature into the static weights
    NF = 512
    NCH = CKK // NF + (1 if CKK % NF else 0)
    wf100 = (f(Wf) / 100.0).astype(ml_dtypes.bfloat16)  # [32, 2304]

    blob_a = np.zeros((P, A_N), np.float32)
    wct = np.ascontiguousarray(f(Wc).T).astype(ml_dtypes.bfloat16)  # [c, o]
    wct_p = wct.reshape(CC, P, C).transpose(1, 0, 2).reshape(P, CC * C)
    blob_a[:, A_WCT0:A_WCT1] = np.ascontiguousarray(wct_p).view(np.float32)
    blob_a[:, A_BC0:A_BC1] = f(bc).reshape(CC, P).T
    wt_p = (
        f(Wt).reshape(CC, P, SQ).transpose(1, 0, 2).reshape(P, CC * SQ)
    ).astype(ml_dtypes.bfloat16)
    blob_a[:, A_WT0:A_WT1] = np.ascontiguousarray(wt_p).view(np.float32)
    blob_a[:SQ, A_BT] = f(bt)
    blob_a[:, A_BCB0:A_BCB1] = (
        (f(bf) / 100.0)
        .reshape(CC, P, KK * KK)
        .transpose(1, 0, 2)
        .reshape(P, CC * KK * KK)
    )

    in_maps = []
    for i in range(BS):
        ba = blob_a.copy()
        tb = (
            temp_feat[i]
            .reshape(CC, P)
            .T.astype(ml_dtypes.bfloat16)
        )  # [128, 2] bf16
        ba[:, A_TMP] = np.ascontiguousarray(tb).view(np.float32)[:, 0]
        in_maps.append({"img": img_pad[i], "bla": ba, "wf": wf100})
    return in_maps


def kernel(image_feat, temp_feat, Wt, bt, Wf, bf, Wc, bc):
    from concourse.bass_utils import run_bass_kernel_spmd

    nc = _get_nc()
    in_maps = _prep_in_maps(image_feat, temp_feat, Wt, bt, Wf, bf, Wc, bc)
    res = run_bass_kernel_spmd(nc, in_maps, core_ids=list(range(BS)))
    _CACHE["last_result"] = res
    out = np.stack([res.results[i]["out"] for i in range(BS)], axis=0)
    return out.astype(np.float32)

